# revision 2
# baseline (speedup 1.0000x reference)
"""Trainium2 Bass kernel: batched dense attention (v2, all-fp16).

Full inputs: queries/keys/values [16, 2048, 64] fp32.
Shards batch dim across 8 NeuronCores (2 batches per core).

Per-core pipeline (batches A, B local):
  S^T[j, q] = K[j, :] . Q[q, :]            (PE fp16, fp32 PSUM, groups of 2)
  P^T = exp(S^T / 8) in fp16, drained CONCURRENTLY by two engines per group:
     ACT: q-columns [0, CA)    exact exp (scale=0.125 fused)
     DVE: q-columns [CA, 512)  Schraudolph bit-trick exp:
          int16(round(s * 0.125 * 1024 * log2e + 15359.5)) bits ~= fp16 exp
  O[q, d] = sum_j P^T[j, q] V'[j, d]       (PE: P^T 128x128 STATIONARY,
          V' [128, 65] moving -> 65-col matmuls; out lands [q, d] -- no
          final transpose. V' 65th col of ones gives the softmax sums.)
  out = O[:, 0:64] * (1 / O[:, 64])        (DVE reciprocal + mult from PSUM)

Performance notes (vs the TimelineSim cost model):
  - The only engines that can read PSUM are ACT (0.83 ns/elem) and DVE
    (1.04 ns/elem); every score element must leave PSUM through one of
    them, so each group is split by column range (CA) and drained by both
    at once, paced with the QK fill.
  - PV uses P^T as the matmul stationary so each MM streams only 65
    columns (D+1) instead of 512; output lands pre-transposed.
  - One flat 64-group software pipeline (3-deep PSUM ring so semaphore
    latency stays hidden); PV matmuls trail QK by 2 groups
    (the PE executes in order -- a PV MM waiting on a drain would block
    the QK MMs queued behind it).
  - PSUM zeroing granularity is a full 2KB bank: only the first matmul
    into each PV bank sets start=True; the other 3 chains' first writes
    rely on the bank-wide pending-zero to act as overwrites.
  - Q^T/K^T are built by PE transposes in batches of 4 per PSUM tile with
    a single merged DVE drain; batches ride the score-group PSUM ring.
"""

import sys
for _p in ("/opt/trn_rl_repo", "/root/.axon_site/_ro/trn_rl_repo"):
    if _p not in sys.path:
        sys.path.insert(0, _p)

import math
import numpy as np

import concourse.bass as bass
import concourse.mybir as mybir
from concourse import bacc
from concourse.tile import TileContext
from concourse.masks import make_identity

F32 = mybir.dt.float32
F16 = mybir.dt.float16
I16 = mybir.dt.int16
P = 128

N_CORES = 8
B_FULL, N, D = 16, 2048, 64
B_LOC = B_FULL // N_CORES          # 2 batches per core
NT = N // P                        # 16 tiles of 128 along q and j
QB = 512                           # q-block
NQB = N // QB                      # 4 q-blocks
QTPB = QB // P                     # 4 q-subtiles per q-block
GROUP = 2                          # S^T slots per PSUM group
SPQ = 2 * NT                       # 32 (j, b) slots per q-block
NGRP = (SPQ + GROUP - 1) // GROUP  # 11 groups per q-block
NGRP_ALL = NQB * NGRP              # 44 groups total
PV_LAG = 2                         # groups between a drain and its PV use

CA = 302                           # ACT takes q-cols [0, CA); DVE the rest
A_DVE = float(0.125 * 1024.0 / math.log(2.0))
B_DVE = float(15 * 1024) - 0.5

_nc_cache = None


def build():
    nc = bacc.Bacc(None, target_bir_lowering=False)
    q_hbm = nc.dram_tensor("queries", [B_LOC, N, D], F32, kind="ExternalInput")
    k_hbm = nc.dram_tensor("keys", [B_LOC, N, D], F32, kind="ExternalInput")
    v_hbm = nc.dram_tensor("values", [B_LOC, N, D], F32, kind="ExternalInput")
    o_hbm = nc.dram_tensor("out", [B_LOC, N, D], F32, kind="ExternalOutput")

    with TileContext(nc) as tc:
        with (
            tc.tile_pool(name="cst", bufs=1) as cst,
            tc.tile_pool(name="stage", bufs=1) as stage,
            tc.tile_pool(name="persist", bufs=1) as persist,
            tc.tile_pool(name="preg", bufs=2) as pregp,
            tc.tile_pool(name="oo", bufs=2) as oop,
            tc.tile_pool(name="st", bufs=3, space="PSUM") as stp,
            tc.tile_pool(name="pv", bufs=2, space="PSUM") as pvp,
        ):
            ident = cst.tile([P, P], F16)
            make_identity(nc, ident)

            # ---- persistent SBUF ----
            # Q^T / K^T, batch-paired: rows 0-63 batch A (d), 64-127 batch B.
            qt = persist.tile([P, N], F16, tag="qt")
            kt = persist.tile([P, N], F16, tag="kt")
            # V' = [V | ones]: [128 j, b, jt, 65] fp16
            v16 = persist.tile([P, B_LOC, NT, D + 1], F16, tag="v16")

            # fp32 staging + fp16 casts, layout [128, t, b, d]
            q_res = [q_hbm[b, :, :].rearrange("(t p) d -> p t d", p=P)
                     for b in range(B_LOC)]
            k_res = [k_hbm[b, :, :].rearrange("(t p) d -> p t d", p=P)
                     for b in range(B_LOC)]
            st32s, st16s = {}, {}
            for name in ("k", "q"):
                st32s[name] = stage.tile([P, NT, B_LOC, D], F32, tag=f"{name}s32",
                                         name=f"{name}s32")
                st16s[name] = stage.tile([P, NT, B_LOC, D], F16, tag=f"{name}s16",
                                         name=f"{name}s16")
            vs32 = stage.tile([P, B_LOC, NT, D], F32, tag="vs32")

            def load_chunk(name, t0, t1, cast_eng):
                re_aps = k_res if name == "k" else q_res
                cs = slice(t0, t1)
                for b in range(B_LOC):
                    nc.sync.dma_start(st32s[name][:, cs, b, :],
                                      re_aps[b][:, cs, :])
                cast_eng.tensor_copy(st16s[name][:, cs], st32s[name][:, cs])

            # Loads: the TimelineSim DMA device is serial, so order = need.
            # All of K must land early (q-block 0 sweeps every j-tile); V and
            # the rest of Q stream in under the running pipeline. V goes out
            # on the ACT HWDGE queue so its issue doesn't delay K/Q on SP.
            # Early casts on the (still idle) DVE; later ones on gpsimd.
            def load_v(t0, t1):
                for b in range(B_LOC):
                    v_re = v_hbm[b, :, :].rearrange("(t p) d -> p t d", p=P)
                    nc.sync.dma_start(vs32[:, b, t0:t1], v_re[:, t0:t1])

            nc.gpsimd.memset(v16[:, :, :, D:D + 1], 1.0)
            load_chunk("k", 0, 4, nc.vector)
            load_chunk("q", 0, 4, nc.vector)
            load_chunk("k", 4, 8, nc.vector)
            load_v(0, 4)
            nc.gpsimd.tensor_copy(v16[:, :, 0:4, 0:D], vs32[:, :, 0:4])
            load_chunk("k", 8, NT, nc.gpsimd)
            load_v(4, NT)
            nc.gpsimd.tensor_copy(v16[:, :, 4:10, 0:D], vs32[:, :, 4:10])
            load_chunk("q", 4, 8, nc.gpsimd)
            nc.gpsimd.tensor_copy(v16[:, :, 10:NT, 0:D], vs32[:, :, 10:NT])
            load_chunk("q", 8, NT, nc.gpsimd)

            # ---- transpose batches: 4 PE transposes -> 1 merged drain.
            def tp_batch(name, t0, drain=None):
                dst = kt if name == "k" else qt
                tp_ps = stp.tile([P, 4, P], F16, tag="st", name=f"tp_{name}{t0}")
                for i in range(4):
                    nc.tensor.transpose(tp_ps[:, i, :], st16s[name][:, t0 + i],
                                        ident[:])
                (drain or nc.vector.tensor_copy)(
                    dst[:, t0 * P:(t0 + 4) * P], tp_ps[:])

            # prep-batch drains ride the ACT engine (idle until the first
            # exp); scalar.copy uses the Copy entry present in every table set
            tp_batch("k", 0, drain=nc.scalar.copy)
            tp_batch("q", 0, drain=nc.scalar.copy)
            tp_batch("k", 4, drain=nc.scalar.copy)
            # remaining batches inside the group loop, timed to data arrival
            # (group g of a q-block consumes kt tile j == g, qt tiles of the
            # q-block; a too-early batch stalls the in-order PE queue):
            late_tps = {6: ("k", 8), 9: ("k", 12), 12: ("q", 4),
                        18: ("q", 8), 26: ("q", 12)}

            # ---- flat software pipeline over all 44 groups ----
            # global group g covers slots [g*3, g*3+3) of q-block g//NGRP;
            # slot s of a q-block: j = s >> 1, b = s & 1.
            preg = {}          # qb -> P region tile
            preg_i = {}
            pv = {}            # (qb, b) -> PSUM accumulator
            next_pv = 0        # global PV step counter (qb*NT + k)

            def emit_group(g):
                qb, gl = divmod(g, NGRP)
                qs = slice(qb * QB, (qb + 1) * QB)
                if gl == 0:
                    preg[qb] = pregp.tile([P, SPQ, QB], F16, tag="preg",
                                          name=f"preg{qb}")
                    preg_i[qb] = preg[qb][:].bitcast(I16)
                st_t = stp.tile([P, GROUP, QB], F32, tag="st",
                                name=f"st{qb}_{gl}")
                for i in range(GROUP):
                    s = gl * GROUP + i
                    j, b = s >> 1, s & 1
                    rows = slice(b * D, (b + 1) * D)
                    nc.tensor.matmul(
                        st_t[:, i, :],
                        kt[rows, j * P:(j + 1) * P],
                        qt[rows, qs],
                        start=True, stop=True,
                    )
                sl = slice(gl * GROUP, (gl + 1) * GROUP)
                nc.scalar.activation(
                    preg[qb][:, sl, 0:CA], st_t[:, :, 0:CA],
                    mybir.ActivationFunctionType.Exp, scale=0.125,
                )
                nc.vector.tensor_scalar(
                    out=preg_i[qb][:, sl, CA:QB], in0=st_t[:, :, CA:QB],
                    scalar1=A_DVE, scalar2=B_DVE,
                    op0=mybir.AluOpType.mult, op1=mybir.AluOpType.add,
                )

            def emit_pv_half(step):
                # one (j-tile, batch) quarter-step: 4 matmuls -- fine grain
                # keeps PE bursts small between QK groups
                qb, rem = divmod(step, 2 * NT)
                k, b = divmod(rem, 2)
                if rem < 2:
                    pv[(qb, b)] = pvp.tile([P, QTPB, P], F32, tag="pv",
                                           name=f"pv{qb}_{b}")
                for t in range(QTPB):
                    # PSUM zeroing granularity is the full 2KB bank: only
                    # each bank's first matmul may set start=True; later
                    # chains' first writes overwrite via the bank-wide
                    # pending-zero (PE runs in program order).
                    nc.tensor.matmul(
                        pv[(qb, b)][:, t, 0:D + 1],
                        preg[qb][:, 2 * k + b, t * P:(t + 1) * P],
                        v16[:, b, k, :],
                        start=(k == 0 and t == 0),
                        stop=(k == NT - 1),
                        skip_group_check=True,
                    )
                if rem == 2 * NT - 1:
                    emit_out_stage(qb)

            def emit_out_stage(qb):
                qs = slice(qb * QB, (qb + 1) * QB)
                for b in range(B_LOC):
                    rec = oop.tile([P, QTPB, 1], F32, tag="rec",
                                   name=f"rec{qb}_{b}")
                    nc.vector.reciprocal(rec[:], pv[(qb, b)][:, :, D:D + 1])
                    o_out = oop.tile([P, QTPB, D], F32, tag="oo",
                                     name=f"oo{qb}_{b}")
                    nc.vector.tensor_tensor(
                        o_out[:], pv[(qb, b)][:, :, 0:D],
                        rec[:].to_broadcast((P, QTPB, D)),
                        mybir.AluOpType.mult,
                    )
                    o_dst = o_hbm[b, qs, :].rearrange("(t p) d -> p t d", p=P)
                    nc.sync.dma_start(o_dst, o_out[:])

            def pv_req_group(step):
                # global group that drains the slot of PV quarter-step `step`
                qb, rem = divmod(step, 2 * NT)
                k, b = divmod(rem, 2)
                return qb * NGRP + (2 * k + b) // GROUP

            for g in range(NGRP_ALL):
                # PV (and any finished q-block's out-stage) goes first so the
                # out-stage precedes the next exps in the DVE queue
                while (next_pv < NQB * NT * 2
                       and pv_req_group(next_pv) <= g - 1 - PV_LAG):
                    emit_pv_half(next_pv)
                    next_pv += 1
                emit_group(g)
                if g in late_tps:
                    tp_batch(*late_tps[g])
            while next_pv < NQB * NT * 2:
                emit_pv_half(next_pv)
                next_pv += 1

    nc.compile()
    return nc


def get_nc():
    global _nc_cache
    if _nc_cache is None:
        _nc_cache = build()
    return _nc_cache


def kernel(queries: np.ndarray, keys: np.ndarray, values: np.ndarray) -> np.ndarray:
    from concourse.bass_utils import run_bass_kernel_spmd

    queries = np.ascontiguousarray(np.asarray(queries, dtype=np.float32))
    keys = np.ascontiguousarray(np.asarray(keys, dtype=np.float32))
    values = np.ascontiguousarray(np.asarray(values, dtype=np.float32))

    nc = get_nc()
    in_maps = []
    for c in range(N_CORES):
        sl = slice(c * B_LOC, (c + 1) * B_LOC)
        in_maps.append({
            "queries": queries[sl],
            "keys": keys[sl],
            "values": values[sl],
        })
    res = run_bass_kernel_spmd(nc, in_maps, core_ids=list(range(N_CORES)))
    return np.concatenate([r["out"] for r in res.results], axis=0)


if __name__ == "__main__":
    rng = np.random.default_rng(0)
    q = rng.standard_normal((B_FULL, N, D), dtype=np.float32)
    k = rng.standard_normal((B_FULL, N, D), dtype=np.float32)
    v = rng.standard_normal((B_FULL, N, D), dtype=np.float32)
    o = kernel(queries=q, keys=k, values=v)
    s = q @ k.transpose(0, 2, 1) / np.sqrt(D)
    w = np.exp(s - s.max(-1, keepdims=True))
    w /= w.sum(-1, keepdims=True)
    ref = w @ v
    err = np.abs(o - ref).max() / np.abs(ref).max()
    print("rel err:", err)


# revision 3
# speedup vs baseline: 1.0062x; 1.0062x over previous
"""Trainium2 Bass kernel: batched dense attention (v2, all-fp16).

Full inputs: queries/keys/values [16, 2048, 64] fp32.
Shards batch dim across 8 NeuronCores (2 batches per core).

Per-core pipeline (batches A, B local):
  S^T[j, q] = K[j, :] . Q[q, :]            (PE fp16, fp32 PSUM, groups of 2)
  P^T = exp(S^T / 8) in fp16, drained CONCURRENTLY by two engines per group:
     ACT: q-columns [0, CA)    exact exp (scale=0.125 fused)
     DVE: q-columns [CA, 512)  Schraudolph bit-trick exp:
          int16(round(s * 0.125 * 1024 * log2e + 15359.5)) bits ~= fp16 exp
  O[q, d] = sum_j P^T[j, q] V'[j, d]       (PE: P^T 128x128 STATIONARY,
          V' [128, 65] moving -> 65-col matmuls; out lands [q, d] -- no
          final transpose. V' 65th col of ones gives the softmax sums.)
  out = O[:, 0:64] * (1 / O[:, 64])        (DVE reciprocal + mult from PSUM)

Performance notes (vs the TimelineSim cost model):
  - The only engines that can read PSUM are ACT (0.83 ns/elem) and DVE
    (1.04 ns/elem); every score element must leave PSUM through one of
    them, so each group is split by column range (CA) and drained by both
    at once, paced with the QK fill.
  - PV uses P^T as the matmul stationary so each MM streams only 65
    columns (D+1) instead of 512; output lands pre-transposed.
  - One flat 64-group software pipeline (3-deep PSUM ring so semaphore
    latency stays hidden); PV matmuls trail QK by 2 groups
    (the PE executes in order -- a PV MM waiting on a drain would block
    the QK MMs queued behind it).
  - PSUM zeroing granularity is a full 2KB bank: only the first matmul
    into each PV bank sets start=True; the other 3 chains' first writes
    rely on the bank-wide pending-zero to act as overwrites.
  - Q^T/K^T are built by PE transposes in batches of 4 per PSUM tile with
    a single merged DVE drain; batches ride the score-group PSUM ring.
"""

import sys
for _p in ("/opt/trn_rl_repo", "/root/.axon_site/_ro/trn_rl_repo"):
    if _p not in sys.path:
        sys.path.insert(0, _p)

import math
import numpy as np

import concourse.bass as bass
import concourse.mybir as mybir
from concourse import bacc
from concourse.tile import TileContext
from concourse.masks import make_identity

F32 = mybir.dt.float32
F16 = mybir.dt.float16
I16 = mybir.dt.int16
P = 128

N_CORES = 8
B_FULL, N, D = 16, 2048, 64
B_LOC = B_FULL // N_CORES          # 2 batches per core
NT = N // P                        # 16 tiles of 128 along q and j
QB = 512                           # q-block
NQB = N // QB                      # 4 q-blocks
QTPB = QB // P                     # 4 q-subtiles per q-block
GROUP = 2                          # S^T slots per PSUM group
SPQ = 2 * NT                       # 32 (j, b) slots per q-block
NGRP = (SPQ + GROUP - 1) // GROUP  # 11 groups per q-block
NGRP_ALL = NQB * NGRP              # 44 groups total
PV_LAG = 2                         # groups between a drain and its PV use

CA = 300                           # ACT takes q-cols [0, CA); DVE the rest
A_DVE = float(0.125 * 1024.0 / math.log(2.0))
B_DVE = float(15 * 1024) - 0.5

_nc_cache = None


def build():
    nc = bacc.Bacc(None, target_bir_lowering=False)
    q_hbm = nc.dram_tensor("queries", [B_LOC, N, D], F32, kind="ExternalInput")
    k_hbm = nc.dram_tensor("keys", [B_LOC, N, D], F32, kind="ExternalInput")
    v_hbm = nc.dram_tensor("values", [B_LOC, N, D], F32, kind="ExternalInput")
    o_hbm = nc.dram_tensor("out", [B_LOC, N, D], F32, kind="ExternalOutput")

    with TileContext(nc) as tc:
        with (
            tc.tile_pool(name="cst", bufs=1) as cst,
            tc.tile_pool(name="stage", bufs=1) as stage,
            tc.tile_pool(name="persist", bufs=1) as persist,
            tc.tile_pool(name="preg", bufs=2) as pregp,
            tc.tile_pool(name="oo", bufs=2) as oop,
            tc.tile_pool(name="st", bufs=3, space="PSUM") as stp,
            tc.tile_pool(name="pv", bufs=2, space="PSUM") as pvp,
        ):
            ident = cst.tile([P, P], F16)
            make_identity(nc, ident)

            # ---- persistent SBUF ----
            # Q^T / K^T, batch-paired: rows 0-63 batch A (d), 64-127 batch B.
            qt = persist.tile([P, N], F16, tag="qt")
            kt = persist.tile([P, N], F16, tag="kt")
            # V' = [V | ones]: [128 j, b, jt, 65] fp16
            v16 = persist.tile([P, B_LOC, NT, D + 1], F16, tag="v16")

            # fp32 staging + fp16 casts, layout [128, t, b, d]
            q_res = [q_hbm[b, :, :].rearrange("(t p) d -> p t d", p=P)
                     for b in range(B_LOC)]
            k_res = [k_hbm[b, :, :].rearrange("(t p) d -> p t d", p=P)
                     for b in range(B_LOC)]
            st32s, st16s = {}, {}
            for name in ("k", "q"):
                st32s[name] = stage.tile([P, NT, B_LOC, D], F32, tag=f"{name}s32",
                                         name=f"{name}s32")
                st16s[name] = stage.tile([P, NT, B_LOC, D], F16, tag=f"{name}s16",
                                         name=f"{name}s16")
            vs32 = stage.tile([P, B_LOC, NT, D], F32, tag="vs32")

            def load_chunk(name, t0, t1, cast_eng):
                re_aps = k_res if name == "k" else q_res
                cs = slice(t0, t1)
                for b in range(B_LOC):
                    nc.sync.dma_start(st32s[name][:, cs, b, :],
                                      re_aps[b][:, cs, :])
                cast_eng.tensor_copy(st16s[name][:, cs], st32s[name][:, cs])

            # Loads: the TimelineSim DMA device is serial, so order = need.
            # All of K must land early (q-block 0 sweeps every j-tile); V and
            # the rest of Q stream in under the running pipeline. V goes out
            # on the ACT HWDGE queue so its issue doesn't delay K/Q on SP.
            # Early casts on the (still idle) DVE; later ones on gpsimd.
            def load_v(t0, t1):
                for b in range(B_LOC):
                    v_re = v_hbm[b, :, :].rearrange("(t p) d -> p t d", p=P)
                    nc.sync.dma_start(vs32[:, b, t0:t1], v_re[:, t0:t1])

            nc.gpsimd.memset(v16[:, :, :, D:D + 1], 1.0)
            load_chunk("k", 0, 4, nc.vector)
            load_chunk("q", 0, 4, nc.vector)
            load_chunk("k", 4, 8, nc.vector)
            load_v(0, 4)
            nc.gpsimd.tensor_copy(v16[:, :, 0:4, 0:D], vs32[:, :, 0:4])
            load_chunk("k", 8, NT, nc.gpsimd)
            load_v(4, NT)
            nc.gpsimd.tensor_copy(v16[:, :, 4:10, 0:D], vs32[:, :, 4:10])
            load_chunk("q", 4, 8, nc.gpsimd)
            nc.gpsimd.tensor_copy(v16[:, :, 10:NT, 0:D], vs32[:, :, 10:NT])
            load_chunk("q", 8, NT, nc.gpsimd)

            # ---- transpose batches: 4 PE transposes -> 1 merged drain.
            def tp_batch(name, t0, drain=None):
                dst = kt if name == "k" else qt
                tp_ps = stp.tile([P, 4, P], F16, tag="st", name=f"tp_{name}{t0}")
                for i in range(4):
                    nc.tensor.transpose(tp_ps[:, i, :], st16s[name][:, t0 + i],
                                        ident[:])
                (drain or nc.vector.tensor_copy)(
                    dst[:, t0 * P:(t0 + 4) * P], tp_ps[:])

            # prep-batch drains ride the ACT engine (idle until the first
            # exp); scalar.copy uses the Copy entry present in every table set
            tp_batch("k", 0, drain=nc.scalar.copy)
            tp_batch("q", 0, drain=nc.scalar.copy)
            tp_batch("k", 4, drain=nc.scalar.copy)
            # remaining batches inside the group loop, timed to data arrival
            # (group g of a q-block consumes kt tile j == g, qt tiles of the
            # q-block; a too-early batch stalls the in-order PE queue):
            late_tps = {6: ("k", 8), 9: ("k", 12), 12: ("q", 4),
                        18: ("q", 8), 26: ("q", 12)}

            # ---- flat software pipeline over all 44 groups ----
            # global group g covers slots [g*3, g*3+3) of q-block g//NGRP;
            # slot s of a q-block: j = s >> 1, b = s & 1.
            preg = {}          # qb -> P region tile
            preg_i = {}
            pv = {}            # (qb, b) -> PSUM accumulator
            next_pv = 0        # global PV step counter (qb*NT + k)

            def emit_group(g):
                qb, gl = divmod(g, NGRP)
                # transpose-batch groups: ACT takes extra columns (it would
                # bubble waiting for the ring anyway) and the DVE -- which
                # also carries the batch drain -- gets fewer
                ca = 490 if g in late_tps else CA
                qs = slice(qb * QB, (qb + 1) * QB)
                if gl == 0:
                    preg[qb] = pregp.tile([P, SPQ, QB], F16, tag="preg",
                                          name=f"preg{qb}")
                    preg_i[qb] = preg[qb][:].bitcast(I16)
                st_t = stp.tile([P, GROUP, QB], F32, tag="st",
                                name=f"st{qb}_{gl}")
                for i in range(GROUP):
                    s = gl * GROUP + i
                    j, b = s >> 1, s & 1
                    rows = slice(b * D, (b + 1) * D)
                    nc.tensor.matmul(
                        st_t[:, i, :],
                        kt[rows, j * P:(j + 1) * P],
                        qt[rows, qs],
                        start=True, stop=True,
                    )
                sl = slice(gl * GROUP, (gl + 1) * GROUP)
                nc.scalar.activation(
                    preg[qb][:, sl, 0:ca], st_t[:, :, 0:ca],
                    mybir.ActivationFunctionType.Exp, scale=0.125,
                )
                nc.vector.tensor_scalar(
                    out=preg_i[qb][:, sl, ca:QB], in0=st_t[:, :, ca:QB],
                    scalar1=A_DVE, scalar2=B_DVE,
                    op0=mybir.AluOpType.mult, op1=mybir.AluOpType.add,
                )

            def emit_pv_half(step):
                # one (j-tile, batch) quarter-step: 4 matmuls -- fine grain
                # keeps PE bursts small between QK groups
                qb, rem = divmod(step, 2 * NT)
                k, b = divmod(rem, 2)
                if rem < 2:
                    pv[(qb, b)] = pvp.tile([P, QTPB, P], F32, tag="pv",
                                           name=f"pv{qb}_{b}")
                for t in range(QTPB):
                    # PSUM zeroing granularity is the full 2KB bank: only
                    # each bank's first matmul may set start=True; later
                    # chains' first writes overwrite via the bank-wide
                    # pending-zero (PE runs in program order).
                    nc.tensor.matmul(
                        pv[(qb, b)][:, t, 0:D + 1],
                        preg[qb][:, 2 * k + b, t * P:(t + 1) * P],
                        v16[:, b, k, :],
                        start=(k == 0 and t == 0),
                        stop=(k == NT - 1),
                        skip_group_check=True,
                    )
                if rem == 2 * NT - 1:
                    emit_out_stage(qb)

            def emit_out_stage(qb):
                qs = slice(qb * QB, (qb + 1) * QB)
                for b in range(B_LOC):
                    rec = oop.tile([P, QTPB, 1], F32, tag="rec",
                                   name=f"rec{qb}_{b}")
                    nc.vector.reciprocal(rec[:], pv[(qb, b)][:, :, D:D + 1])
                    o_out = oop.tile([P, QTPB, D], F32, tag="oo",
                                     name=f"oo{qb}_{b}")
                    nc.vector.tensor_tensor(
                        o_out[:], pv[(qb, b)][:, :, 0:D],
                        rec[:].to_broadcast((P, QTPB, D)),
                        mybir.AluOpType.mult,
                    )
                    o_dst = o_hbm[b, qs, :].rearrange("(t p) d -> p t d", p=P)
                    nc.sync.dma_start(o_dst, o_out[:])

            def pv_req_group(step):
                # global group that drains the slot of PV quarter-step `step`
                qb, rem = divmod(step, 2 * NT)
                k, b = divmod(rem, 2)
                return qb * NGRP + (2 * k + b) // GROUP

            for g in range(NGRP_ALL):
                # PV (and any finished q-block's out-stage) goes first so the
                # out-stage precedes the next exps in the DVE queue
                while (next_pv < NQB * NT * 2
                       and pv_req_group(next_pv) <= g - 1 - PV_LAG):
                    emit_pv_half(next_pv)
                    next_pv += 1
                emit_group(g)
                if g in late_tps:
                    tp_batch(*late_tps[g])
            while next_pv < NQB * NT * 2:
                emit_pv_half(next_pv)
                next_pv += 1

    nc.compile()
    return nc


def get_nc():
    global _nc_cache
    if _nc_cache is None:
        _nc_cache = build()
    return _nc_cache


def kernel(queries: np.ndarray, keys: np.ndarray, values: np.ndarray) -> np.ndarray:
    from concourse.bass_utils import run_bass_kernel_spmd

    queries = np.ascontiguousarray(np.asarray(queries, dtype=np.float32))
    keys = np.ascontiguousarray(np.asarray(keys, dtype=np.float32))
    values = np.ascontiguousarray(np.asarray(values, dtype=np.float32))

    nc = get_nc()
    in_maps = []
    for c in range(N_CORES):
        sl = slice(c * B_LOC, (c + 1) * B_LOC)
        in_maps.append({
            "queries": queries[sl],
            "keys": keys[sl],
            "values": values[sl],
        })
    res = run_bass_kernel_spmd(nc, in_maps, core_ids=list(range(N_CORES)))
    return np.concatenate([r["out"] for r in res.results], axis=0)


if __name__ == "__main__":
    rng = np.random.default_rng(0)
    q = rng.standard_normal((B_FULL, N, D), dtype=np.float32)
    k = rng.standard_normal((B_FULL, N, D), dtype=np.float32)
    v = rng.standard_normal((B_FULL, N, D), dtype=np.float32)
    o = kernel(queries=q, keys=k, values=v)
    s = q @ k.transpose(0, 2, 1) / np.sqrt(D)
    w = np.exp(s - s.max(-1, keepdims=True))
    w /= w.sum(-1, keepdims=True)
    ref = w @ v
    err = np.abs(o - ref).max() / np.abs(ref).max()
    print("rel err:", err)
